# revision 4
# baseline (speedup 1.0000x reference)
"""Cosine-similarity attention map on 8 Trainium2 NeuronCores.

out[b, i, j] = <x[b,:,i], x[b,:,j]> / (||x[b,:,i]|| * ||x[b,:,j]||)
x: [B=4, C=64, N=4096] fp32  ->  out: [B=4, N=4096, N=4096] fp32

The output is a symmetric Gram matrix per batch, so each core only
computes a unique half of it (SYRK-style) and the host mirrors the rest
while unsharding. Sharding: 2 cores per batch running the SAME program;
core (b, 0) gets x[b], core (b, 1) gets x[b] with columns reversed.
In its own index space every core computes, for each 128-row tile
a in [0,16): cols [128a, 2048) (triangle part) and cols
[3968-128a, 4096) (anti-diagonal cross part) -- a constant 2176
columns per tile. The identity-core blocks plus the mirrored
reversed-core blocks tile the full matrix exactly once (plus the 16
anti-diagonal blocks twice). Output is written as fp16 (tolerance is
2e-2; fp16 adds ~3e-4) and upcast on the host, so HBM write traffic is
8.9 MiB/core instead of 32 MiB/core.

Engine budget per core (PE is the wall at the observed ~0.83 ns/row
sustained): PE = 34816 SYRK rows + 4096 norm-reduce rows. The norm
reduce+broadcast is ONE matmul per 512 cols (lhsT = ones[64,64] gives
column sums broadcast to all 64 partitions); square and y=x*rsqrt run
on the otherwise-idle GpSimd engine; reciprocal on DVE, sqrt on ACT.
PSUM->SBUF copies are 1024-wide (pairs of banks) alternating DVE/ACT.
"""

import sys

sys.path.insert(0, "/opt/trn_rl_repo")

import numpy as np

import concourse.bass as bass
import concourse.mybir as mybir
import concourse.tile as tile
from concourse import bacc
from concourse.bass_utils import run_bass_kernel_spmd
from concourse.vector_clock import ScopedClock, VectorClock

B, C, N = 4, 64, 4096
NCORES = 8
NTILES = 16  # 128-row output tiles per core
TW = 2176  # output columns per row tile (constant by construction)
UW = 2 * TW  # two tiles share one 128-partition DMA panel

F32 = mybir.dt.float32
F16 = mybir.dt.float16
Sqrt = mybir.ActivationFunctionType.Sqrt


class SplitDrainTileContext(tile.TileContext):
    """Stock TileContext attaches a wait for every pending DMA-queue
    semaphore to a single exit Drain; the walrus build here only allows one
    sync-wait per TPB_CTRL instruction ("Too many sync wait commands").
    Emit one drain per pending logical processor instead."""

    def _drain_and_barrier(self, tick_clock, wait_clock):
        gc = tick_clock.global_clock
        n = len(gc)
        for p in range(n):
            t = gc[p]
            if t <= 0:
                continue
            part = VectorClock([t if q == p else 0 for q in range(n)])
            d = self.nc.sync.drain()
            wait_clock.add_sem_waits(d.ins, ScopedClock({None: part}))

        self.nc.all_engine_barrier()
        assert self.sems is not None
        popped = self.nc._tile_sem_poison_stack.pop()
        assert popped is self._sem_poison
        self.nc.clear_and_free_semaphores(list(self.sems.allocated().values()))
        self.nc.all_engine_barrier()


def _ranges(a):
    """(start, width) column ranges of row tile a: triangle + cross part."""
    return [(128 * a, 2048 - 128 * a), (3968 - 128 * a, 128 + 128 * a)]


def _pack(ranges):
    """Pack matmul chunks into [128,1024] PSUM pair-tiles.

    Returns groups: (mms, glen) where mms = [(rhs_start, width, slot_off)].
    Each matmul stays within one 512-col PSUM bank; each group is copied
    to SBUF with a single contiguous [0:glen) copy.
    """
    groups, cur, cur_len = [], [], 0
    for start, width in ranges:
        done = 0
        while done < width:
            if cur_len == 1024:
                groups.append((cur, cur_len))
                cur, cur_len = [], 0
            w = min(512 - (cur_len % 512), width - done)
            cur.append((start + done, w, cur_len))
            cur_len += w
            done += w
    if cur:
        groups.append((cur, cur_len))
    return groups


def _build(use_split_drain=False):
    nc = bacc.Bacc("TRN2", target_bir_lowering=False)
    xf = nc.declare_dram_parameter("xf", [C, N], F32, isOutput=False)
    out = nc.declare_dram_parameter("out", [NTILES // 2 * 128, UW], F16, isOutput=True)

    tc_cls = SplitDrainTileContext if use_split_drain else tile.TileContext
    with tc_cls(nc) as tc:
        with (
            tc.tile_pool(name="persist", bufs=1) as persist,
            tc.tile_pool(name="panels", bufs=3) as panels,
            tc.tile_pool(name="nsb", bufs=2) as nsb,
            tc.tile_pool(name="mpsum", bufs=3, space="PSUM") as mpsum,
            tc.tile_pool(name="npsum", bufs=1, space="PSUM") as npsum,
        ):
            XF = persist.tile([C, N], F32)
            for q in range(4):
                nc.sync.dma_start(
                    out=XF[:, 1024 * q : 1024 * (q + 1)],
                    in_=xf[:, 1024 * q : 1024 * (q + 1)],
                )

            ones_f = persist.tile([C, C], F32)
            nc.vector.memset(ones_f, 1.0)
            ones_J = persist.tile([C, C], F16)  # reduce+broadcast lhsT
            nc.vector.tensor_copy(ones_J, ones_f)

            SQ = persist.tile([C, N], F16)
            RN = persist.tile([C, N], F16)
            YF = persist.tile([C, N], F16)

            def norm_pair(p):
                # Normalize cols [1024p, 1024p+1024): square (GpSimd) ->
                # ones[64,64]-matmul = column sums broadcast to all 64
                # partitions (PE) -> approx reciprocal (DVE) -> sqrt to
                # fp16 (ACT) -> y = x * rsqrt (GpSimd).
                cs = slice(1024 * p, 1024 * (p + 1))
                nc.gpsimd.tensor_mul(SQ[:, cs], XF[:, cs], XF[:, cs])
                pj = npsum.tile([C, 1024], F32, tag="pj")
                for h in range(2):
                    hs = slice(512 * h, 512 * h + 512)
                    s0 = 1024 * p + 512 * h
                    nc.tensor.matmul(
                        pj[:, hs],
                        lhsT=ones_J,
                        rhs=SQ[:, s0 : s0 + 512],
                        start=True,
                        stop=True,
                    )
                rsb = nsb.tile([C, 1024], F32, tag="rsb")
                nc.vector.reciprocal_approx_fast(rsb, pj)
                nc.scalar.activation(RN[:, cs], rsb, Sqrt)
                nc.gpsimd.tensor_mul(YF[:, cs], XF[:, cs], RN[:, cs])

            ncopy = 0

            def emit_groups(a, groups, panel, off):
                # SYRK matmuls for row tile a, packed into PSUM pair-tiles;
                # one contiguous PSUM->SBUF fp16 copy per group, alternating
                # DVE / ACT.
                nonlocal ncopy
                lhsT = YF[:, 128 * a : 128 * a + 128]
                for mms, glen in groups:
                    ps = mpsum.tile([128, 1024], F32, tag="ps")
                    for rs, w, so in mms:
                        nc.tensor.matmul(
                            ps[:, so : so + w],
                            lhsT=lhsT,
                            rhs=YF[:, rs : rs + w],
                            start=True,
                            stop=True,
                        )
                    if ncopy % 2 == 0:
                        nc.vector.tensor_copy(panel[:, off : off + glen], ps[:, :glen])
                    else:
                        nc.scalar.copy(out=panel[:, off : off + glen], in_=ps[:, :glen])
                    ncopy += 1
                    off += glen

            # Norm cols [0,2048) first; tiles 0-3 triangle matmuls overlap
            # with the [2048,4096) norm; cross parts (which need the last
            # norm chunk) follow; then the remaining tiles flow through.
            norm_pair(0)
            norm_pair(1)
            early0 = panels.tile([128, UW], F16, tag="panel")
            early1 = panels.tile([128, UW], F16, tag="panel")
            early = [early0, early1]
            for a in range(4):
                u, s = divmod(a, 2)
                r1, r2 = _ranges(a)
                emit_groups(a, _pack([r1]), early[u], s * TW)
            norm_pair(2)
            norm_pair(3)
            for a in range(4):
                u, s = divmod(a, 2)
                r1, r2 = _ranges(a)
                emit_groups(a, _pack([r2]), early[u], s * TW + r1[1])
            for u in range(2):
                nc.sync.dma_start(
                    out=out[u * 128 : (u + 1) * 128, :], in_=early[u]
                )

            for u in range(2, NTILES // 2):
                panel = panels.tile([128, UW], F16, tag="panel")
                for s in range(2):
                    a = 2 * u + s
                    emit_groups(a, _pack(_ranges(a)), panel, s * TW)
                nc.sync.dma_start(out=out[u * 128 : (u + 1) * 128, :], in_=panel)

    nc.compile()
    return nc


def _install_profile_hook():
    """This container's antenv lacks axon_hooks, so run_bass_kernel_spmd's
    trace=True path dies on import. Recreate the module and register the
    ctypes NTFF hook that trn_boot would have installed."""
    import sys as _sys
    import types

    if "antenv.axon_hooks" in _sys.modules:
        return
    import antenv

    mod = types.ModuleType("antenv.axon_hooks")
    mod._hook = None

    def set_axon_ntff_profile_hook(h):
        mod._hook = h

    def get_axon_ntff_profile_hook():
        return mod._hook

    mod.set_axon_ntff_profile_hook = set_axon_ntff_profile_hook
    mod.get_axon_ntff_profile_hook = get_axon_ntff_profile_hook
    _sys.modules["antenv.axon_hooks"] = mod
    antenv.axon_hooks = mod

    from trn_agent_boot.trn_boot import _ntff_profile_via_ctypes

    mod.set_axon_ntff_profile_hook(
        _ntff_profile_via_ctypes("/opt/axon/libaxon_pjrt.so")
    )


_nc = None


def _get_nc():
    global _nc
    if _nc is None:
        _nc = _build()
    return _nc


# Ordered output blocks (32x32 grid of 128x128) filled by the two cores
# of a batch; the rest is mirrored from the transpose on the host.
_FILLED = np.zeros((32, 32), bool)
for _a in range(16):
    _FILLED[_a, _a:16] = True
    _FILLED[_a, 31 - _a : 32] = True
    _FILLED[31 - _a, 16 : 32 - _a] = True
    _FILLED[31 - _a, 0 : _a + 1] = True
_MISS_I, _MISS_J = np.nonzero(~_FILLED)


def _run(x, trace=False, trace_cores=None):
    x = np.asarray(x, dtype=np.float32)
    assert x.shape == (B, C, N), x.shape
    core_ids = list(range(NCORES))
    in_maps = []
    for k in core_ids:
        b, half = divmod(k, 2)
        xb = x[b] if half == 0 else x[b][:, ::-1]
        in_maps.append({"xf": np.ascontiguousarray(xb)})
    if trace:
        _install_profile_hook()
    res = run_bass_kernel_spmd(
        _get_nc(), in_maps, core_ids, trace=trace, trace_cores=trace_cores
    )
    out = np.empty((B, N, N), dtype=np.float32)
    for k in core_ids:
        b, half = divmod(k, 2)
        O = res.results[k]["out"]
        M = out[b]
        for a in range(NTILES):
            u, s = divmod(a, 2)
            P = O[u * 128 : (u + 1) * 128, s * TW : (s + 1) * TW]
            W1 = 2048 - 128 * a
            if half == 0:
                M[128 * a : 128 * a + 128, 128 * a : 2048] = P[:, :W1]
                M[128 * a : 128 * a + 128, 3968 - 128 * a : 4096] = P[:, W1:]
            else:
                M[3968 - 128 * a : 4096 - 128 * a, 2048 : 4096 - 128 * a] = P[
                    :, :W1
                ][::-1, ::-1]
                M[3968 - 128 * a : 4096 - 128 * a, 0 : 128 * a + 128] = P[:, W1:][
                    ::-1, ::-1
                ]
    for b in range(B):
        Mb = out[b].reshape(32, 128, 32, 128)
        Mb[_MISS_I, :, _MISS_J, :] = Mb[_MISS_J, :, _MISS_I, :].transpose(0, 2, 1)
    return out, res


def kernel(x):
    return _run(x)[0]


# revision 8
# speedup vs baseline: 1.1129x; 1.1129x over previous
"""Cosine-similarity attention map on 8 Trainium2 NeuronCores.

out[b, i, j] = <x[b,:,i], x[b,:,j]> / (||x[b,:,i]|| * ||x[b,:,j]||)
x: [B=4, C=64, N=4096] fp32  ->  out: [B=4, N=4096, N=4096] fp32

The output is a symmetric Gram matrix per batch, so each core only
computes a unique half of it (SYRK-style) and the host mirrors the rest
while unsharding. Sharding: 2 cores per batch running the SAME program;
core (b, 0) gets x[b], core (b, 1) gets x[b] with columns reversed.
In its own index space every core computes, for each 128-row tile
a in [0,16): cols [128a, 2048) (triangle part) and cols
[3968-128a, 4096) (anti-diagonal cross part) -- a constant 2176
columns per tile. The identity-core blocks plus the mirrored
reversed-core blocks tile the full matrix exactly once (plus the 16
anti-diagonal blocks twice). Output is written as fp16 (tolerance is
2e-2; fp16 adds ~3e-4) and upcast on the host, so HBM write traffic is
8.9 MiB/core instead of 32 MiB/core.

Engine budget per core (PE is the wall at the observed ~0.83 ns/row
sustained): PE = 34816 SYRK rows + 4096 norm-reduce rows. The norm
reduce+broadcast is ONE matmul per 512 cols (lhsT = ones[64,64] gives
column sums broadcast to all 64 partitions); square and y=x*rsqrt run
on the otherwise-idle GpSimd engine; reciprocal on DVE, sqrt on ACT.
PSUM->SBUF copies are 1024-wide (pairs of banks) alternating DVE/ACT.
"""

import sys

sys.path.insert(0, "/opt/trn_rl_repo")

import numpy as np

import concourse.bass as bass
import concourse.mybir as mybir
import concourse.tile as tile
from concourse import bacc
from concourse.bass_utils import run_bass_kernel_spmd
from concourse.vector_clock import ScopedClock, VectorClock

B, C, N = 4, 64, 4096
NCORES = 8
NTILES = 16  # 128-row output tiles per core
TW = 2176  # output columns per row tile (constant by construction)
UW = 2 * TW  # two tiles share one 128-partition DMA panel

F32 = mybir.dt.float32
F16 = mybir.dt.float16
AbsRsqrt = mybir.ActivationFunctionType.Abs_reciprocal_sqrt


class SplitDrainTileContext(tile.TileContext):
    """Stock TileContext attaches a wait for every pending DMA-queue
    semaphore to a single exit Drain; the walrus build here only allows one
    sync-wait per TPB_CTRL instruction ("Too many sync wait commands").
    Emit one drain per pending logical processor instead."""

    def _drain_and_barrier(self, tick_clock, wait_clock):
        gc = tick_clock.global_clock
        n = len(gc)
        for p in range(n):
            t = gc[p]
            if t <= 0:
                continue
            part = VectorClock([t if q == p else 0 for q in range(n)])
            d = self.nc.sync.drain()
            wait_clock.add_sem_waits(d.ins, ScopedClock({None: part}))

        self.nc.all_engine_barrier()
        assert self.sems is not None
        popped = self.nc._tile_sem_poison_stack.pop()
        assert popped is self._sem_poison
        self.nc.clear_and_free_semaphores(list(self.sems.allocated().values()))
        self.nc.all_engine_barrier()


def _ranges(a):
    """(start, width) column ranges of row tile a: triangle + cross part."""
    return [(128 * a, 2048 - 128 * a), (3968 - 128 * a, 128 + 128 * a)]


def _pack(ranges):
    """Pack matmul chunks into [128,1024] PSUM pair-tiles.

    Returns groups: (mms, glen) where mms = [(rhs_start, width, slot_off)].
    Each matmul stays within one 512-col PSUM bank; each group is copied
    to SBUF with a single contiguous [0:glen) copy.
    """
    groups, cur, cur_len = [], [], 0
    for start, width in ranges:
        done = 0
        while done < width:
            if cur_len == 1024:
                groups.append((cur, cur_len))
                cur, cur_len = [], 0
            w = min(512 - (cur_len % 512), width - done)
            cur.append((start + done, w, cur_len))
            cur_len += w
            done += w
    if cur:
        groups.append((cur, cur_len))
    return groups


def _build(use_split_drain=False):
    nc = bacc.Bacc("TRN2", target_bir_lowering=False)
    xf = nc.declare_dram_parameter("xf", [C, N], F32, isOutput=False)
    out = nc.declare_dram_parameter("out", [NTILES // 2 * 128, UW], F16, isOutput=True)

    tc_cls = SplitDrainTileContext if use_split_drain else tile.TileContext
    with tc_cls(nc) as tc:
        with (
            tc.tile_pool(name="persist", bufs=1) as persist,
            tc.tile_pool(name="panels", bufs=3) as panels,
            tc.tile_pool(name="mpsum", bufs=4, space="PSUM") as mpsum,
        ):
            XF = persist.tile([C, N], F32)
            for q in range(4):
                nc.sync.dma_start(
                    out=XF[:, 1024 * q : 1024 * (q + 1)],
                    in_=xf[:, 1024 * q : 1024 * (q + 1)],
                )

            ones_f = persist.tile([C, C], F32)
            nc.vector.memset(ones_f, 1.0)
            ones_J = persist.tile([C, C], F16)  # reduce+broadcast lhsT
            nc.vector.tensor_copy(ones_J, ones_f)

            SQ = persist.tile([C, N], F16)
            RN = persist.tile([C, N], F16)
            YF = persist.tile([C, N], F16)

            def norm_pair(p):
                # Normalize cols [1024p, 1024p+1024) in two 512 chunks:
                # square (round-robin DVE/Pool/ACT) -> ones[64,64]-matmul =
                # column sums broadcast to all 64 partitions (PE) ->
                # 1/sqrt via Abs_reciprocal_sqrt straight off PSUM to fp16
                # (ACT) -> y = x * rsqrt (DVE).
                pj = mpsum.tile([128, 1024], F32, tag="ps")
                for h in range(2):
                    c = 2 * p + h
                    cs = slice(512 * c, 512 * (c + 1))
                    if c % 3 == 2:
                        nc.scalar.square(SQ[:, cs], XF[:, cs])
                    else:
                        sq_eng = (nc.vector, nc.gpsimd)[c % 3]
                        sq_eng.tensor_mul(SQ[:, cs], XF[:, cs], XF[:, cs])
                    hs = slice(512 * h, 512 * h + 512)
                    nc.tensor.matmul(
                        pj[0:C, hs],
                        lhsT=ones_J,
                        rhs=SQ[:, cs],
                        start=True,
                        stop=True,
                    )
                    nc.scalar.activation(RN[:, cs], pj[0:C, hs], AbsRsqrt)
                    nc.vector.tensor_mul(YF[:, cs], XF[:, cs], RN[:, cs])

            ncopy = 0

            def emit_groups(a, groups, panel, off):
                # SYRK matmuls for row tile a, packed into PSUM pair-tiles;
                # one contiguous PSUM->SBUF fp16 copy per group, alternating
                # DVE / ACT.
                nonlocal ncopy
                lhsT = YF[:, 128 * a : 128 * a + 128]
                for mms, glen in groups:
                    ps = mpsum.tile([128, 1024], F32, tag="ps")
                    for rs, w, so in mms:
                        nc.tensor.matmul(
                            ps[:, so : so + w],
                            lhsT=lhsT,
                            rhs=YF[:, rs : rs + w],
                            start=True,
                            stop=True,
                        )
                    if ncopy % 2 == 0:
                        nc.vector.tensor_copy(panel[:, off : off + glen], ps[:, :glen])
                    else:
                        nc.scalar.copy(out=panel[:, off : off + glen], in_=ps[:, :glen])
                    ncopy += 1
                    off += glen

            # Norm cols [0,2048) first; tiles 0-3 triangle matmuls overlap
            # with the [2048,4096) norm; cross parts (which need the last
            # norm chunk) follow; then the remaining tiles flow through.
            norm_pair(0)
            norm_pair(1)
            early0 = panels.tile([128, UW], F16, tag="panel")
            early1 = panels.tile([128, UW], F16, tag="panel")
            early = [early0, early1]
            for a in range(4):
                u, s = divmod(a, 2)
                r1, r2 = _ranges(a)
                emit_groups(a, _pack([r1]), early[u], s * TW)
            norm_pair(2)
            norm_pair(3)
            for a in range(4):
                u, s = divmod(a, 2)
                r1, r2 = _ranges(a)
                emit_groups(a, _pack([r2]), early[u], s * TW + r1[1])
                nc.sync.dma_start(
                    out=out[u * 128 : (u + 1) * 128, s * TW : (s + 1) * TW],
                    in_=early[u][:, s * TW : (s + 1) * TW],
                )

            for u in range(2, NTILES // 2):
                panel = panels.tile([128, UW], F16, tag="panel")
                for s in range(2):
                    a = 2 * u + s
                    emit_groups(a, _pack(_ranges(a)), panel, s * TW)
                    nc.sync.dma_start(
                        out=out[u * 128 : (u + 1) * 128, s * TW : (s + 1) * TW],
                        in_=panel[:, s * TW : (s + 1) * TW],
                    )

    nc.compile()
    return nc


def _install_profile_hook():
    """This container's antenv lacks axon_hooks, so run_bass_kernel_spmd's
    trace=True path dies on import. Recreate the module and register the
    ctypes NTFF hook that trn_boot would have installed."""
    import sys as _sys
    import types

    if "antenv.axon_hooks" in _sys.modules:
        return
    import antenv

    mod = types.ModuleType("antenv.axon_hooks")
    mod._hook = None

    def set_axon_ntff_profile_hook(h):
        mod._hook = h

    def get_axon_ntff_profile_hook():
        return mod._hook

    mod.set_axon_ntff_profile_hook = set_axon_ntff_profile_hook
    mod.get_axon_ntff_profile_hook = get_axon_ntff_profile_hook
    _sys.modules["antenv.axon_hooks"] = mod
    antenv.axon_hooks = mod

    from trn_agent_boot.trn_boot import _ntff_profile_via_ctypes

    mod.set_axon_ntff_profile_hook(
        _ntff_profile_via_ctypes("/opt/axon/libaxon_pjrt.so")
    )


_nc = None


def _get_nc():
    global _nc
    if _nc is None:
        _nc = _build()
    return _nc


# Ordered output blocks (32x32 grid of 128x128) filled by the two cores
# of a batch; the rest is mirrored from the transpose on the host.
_FILLED = np.zeros((32, 32), bool)
for _a in range(16):
    _FILLED[_a, _a:16] = True
    _FILLED[_a, 31 - _a : 32] = True
    _FILLED[31 - _a, 16 : 32 - _a] = True
    _FILLED[31 - _a, 0 : _a + 1] = True
_MISS_I, _MISS_J = np.nonzero(~_FILLED)


def _run(x, trace=False, trace_cores=None):
    x = np.asarray(x, dtype=np.float32)
    assert x.shape == (B, C, N), x.shape
    core_ids = list(range(NCORES))
    in_maps = []
    for k in core_ids:
        b, half = divmod(k, 2)
        xb = x[b] if half == 0 else x[b][:, ::-1]
        in_maps.append({"xf": np.ascontiguousarray(xb)})
    if trace:
        _install_profile_hook()
    res = run_bass_kernel_spmd(
        _get_nc(), in_maps, core_ids, trace=trace, trace_cores=trace_cores
    )
    out = np.empty((B, N, N), dtype=np.float32)
    for k in core_ids:
        b, half = divmod(k, 2)
        O = res.results[k]["out"]
        M = out[b]
        for a in range(NTILES):
            u, s = divmod(a, 2)
            P = O[u * 128 : (u + 1) * 128, s * TW : (s + 1) * TW]
            W1 = 2048 - 128 * a
            if half == 0:
                M[128 * a : 128 * a + 128, 128 * a : 2048] = P[:, :W1]
                M[128 * a : 128 * a + 128, 3968 - 128 * a : 4096] = P[:, W1:]
            else:
                M[3968 - 128 * a : 4096 - 128 * a, 2048 : 4096 - 128 * a] = P[
                    :, :W1
                ][::-1, ::-1]
                M[3968 - 128 * a : 4096 - 128 * a, 0 : 128 * a + 128] = P[:, W1:][
                    ::-1, ::-1
                ]
    for b in range(B):
        Mb = out[b].reshape(32, 128, 32, 128)
        Mb[_MISS_I, :, _MISS_J, :] = Mb[_MISS_J, :, _MISS_I, :].transpose(0, 2, 1)
    return out, res


def kernel(x):
    return _run(x)[0]
